# revision 2
# baseline (speedup 1.0000x reference)
"""Bass/Trainium2 kernel for nn_BilinearPairedLayer.

Math (per batch b):
  xl = concat([x, shift_down(x,1), shift_up(x,1)], -1)      # [N, 192]
  xr = concat([x, shift_up(x,1), shift_down(x,1)], -1)
  hl = relu(xl @ W_l.T + b_l)                               # [N, 128]
  hr = relu(xr @ W_r.T + b_r)
  out[i,j,k] = sum_g (hl @ W_bil[k])[i,g] * hr[j,g] + b_bil[k]   # [N, N, 2]

Sharding: data-parallel over B — core c computes batch b=c (B=8, 8 cores).

Per-core dataflow (all fp32):
  - transpose x -> xT [64, N] via PE transposes (8 tiles of [128, 64])
  - the context shifts are free: the shifted feature chunks of xl^T are just
    column-offset views of xT, so hl^T accumulates 3 matmuls per N-chunk
    with restricted column ranges at the sequence edges
  - hlT/hrT [128, N] = relu(W @ xlT + b) with per-partition ACT bias
  - tTk [128, N] = W_bil[k] (stationary, as stored) @ hlT
  - out tile [i=128, j=512] = tTk[:, iblk].T @ hrT chunk  (PSUM, contiguous)
  - bias + (j,k)-interleave fused into the mandatory PSUM->SBUF copy:
    DVE tensor_scalar_add writes k=0 stride-2 columns, ACT activation
    (Identity, per-partition bias) writes k=1 — then one 1 MiB DMA per
    row-block of 128 i values.
"""

import numpy as np

B, N, NIN = 8, 1024, 64
H = 128
NOUT = 2
NCH = 512  # matmul free-dim chunk (one PSUM bank of fp32)

_cached = {}


def _build():
    import concourse.bacc as bacc
    import concourse.mybir as mybir
    import concourse.tile as tile
    from concourse.masks import make_identity

    f32 = mybir.dt.float32
    AF = mybir.ActivationFunctionType

    nc = bacc.Bacc("TRN2", target_bir_lowering=False, debug=False, num_devices=8)

    x_d = nc.dram_tensor("x", [N, NIN], f32, kind="ExternalInput").ap()
    wl_d = nc.dram_tensor("w_l", [H, 3 * NIN], f32, kind="ExternalInput").ap()
    bl_d = nc.dram_tensor("b_l", [H], f32, kind="ExternalInput").ap()
    wr_d = nc.dram_tensor("w_r", [H, 3 * NIN], f32, kind="ExternalInput").ap()
    br_d = nc.dram_tensor("b_r", [H], f32, kind="ExternalInput").ap()
    wb_d = nc.dram_tensor("w_bil", [NOUT, H, H], f32, kind="ExternalInput").ap()
    bb_d = nc.dram_tensor("b_bil", [NOUT], f32, kind="ExternalInput").ap()
    out_d = nc.dram_tensor("out", [N, N, NOUT], f32, kind="ExternalOutput").ap()
    # row-block view: [8 blocks, 128 rows, (j,k) interleaved 2048]
    out_v = out_d.rearrange("(t p) n k -> t p (n k)", p=128)

    with tile.TileContext(nc) as tc:
        with (
            tc.tile_pool(name="const", bufs=1) as const,
            tc.tile_pool(name="ld", bufs=2) as ld,
            tc.tile_pool(name="ps", bufs=8, space="PSUM") as ps,
            tc.tile_pool(name="ob", bufs=3) as ob,
        ):
            ident = const.tile([128, 128], f32)
            make_identity(nc, ident)

            wl_s = const.tile([H, 3 * NIN], f32)
            nc.sync.dma_start(out=wl_s, in_=wl_d)
            wr_s = const.tile([H, 3 * NIN], f32)
            nc.sync.dma_start(out=wr_s, in_=wr_d)
            wb0 = const.tile([H, H], f32)
            nc.sync.dma_start(out=wb0, in_=wb_d[0])
            wb1 = const.tile([H, H], f32)
            nc.sync.dma_start(out=wb1, in_=wb_d[1])
            bl_s = const.tile([H, 1], f32)
            nc.sync.dma_start(out=bl_s, in_=bl_d.unsqueeze(1))
            br_s = const.tile([H, 1], f32)
            nc.sync.dma_start(out=br_s, in_=br_d.unsqueeze(1))
            bb_s = const.tile([128, NOUT], f32)
            nc.gpsimd.dma_start(out=bb_s, in_=bb_d.unsqueeze(0).broadcast_to([128, NOUT]))

            # ---- transpose x -> xT [64, 1024]
            xload = ld.tile([128, 8, NIN], f32)
            nc.sync.dma_start(out=xload, in_=x_d.rearrange("(t p) f -> p t f", p=128))
            xT = const.tile([NIN, N], f32)
            for t in range(8):
                pst = ps.tile([128, NCH], f32, tag="ps")
                nc.tensor.transpose(pst[0:NIN, 0:128], xload[:, t, :], ident)
                nc.scalar.copy(xT[:, t * 128 : (t + 1) * 128], pst[0:NIN, 0:128])

            # ---- transpose W chunks -> lhsT layout [64, 128] each
            wlT = const.tile([NIN, 3, H], f32)
            wrT = const.tile([NIN, 3, H], f32)
            for wt, ws in ((wlT, wl_s), (wrT, wr_s)):
                for c in range(3):
                    pst = ps.tile([128, NCH], f32, tag="ps")
                    nc.tensor.transpose(
                        pst[0:NIN, 0:H], ws[:, c * NIN : (c + 1) * NIN], ident
                    )
                    nc.scalar.copy(wt[:, c, :], pst[0:NIN, 0:H])

            # ---- hlT / hrT = relu(W @ xlT + b), shifts as column offsets
            hlT = const.tile([H, N], f32)
            hrT = const.tile([H, N], f32)
            # chunk 1 is shift_down (src col i-1) for xl, shift_up (i+1) for xr
            for dst, wt, bias, s1 in ((hlT, wlT, bl_s, -1), (hrT, wrT, br_s, +1)):
                for j0 in (0, NCH):
                    ph = ps.tile([128, NCH], f32, tag="ps")
                    nc.tensor.matmul(
                        ph[:, :], wt[:, 0, :], xT[:, j0 : j0 + NCH],
                        start=True, stop=False,
                    )
                    for c, s in ((1, s1), (2, -s1)):
                        last = c == 2
                        lo = max(j0, -s)          # first valid out col
                        hi = min(j0 + NCH, N - s) # one past last valid out col
                        nc.tensor.matmul(
                            ph[:, lo - j0 : hi - j0],
                            wt[:, c, :],
                            xT[:, lo + s : hi + s],
                            start=False, stop=last,
                        )
                    nc.scalar.activation(
                        dst[:, j0 : j0 + NCH], ph[:, :], AF.Relu,
                        bias=bias[:, 0:1], scale=1.0,
                    )

            # ---- tTk = W_bil[k] @ hlT  (lhsT = W_bil[k] as stored)
            tT0 = const.tile([H, N], f32)
            tT1 = const.tile([H, N], f32)
            for wb, tT in ((wb0, tT0), (wb1, tT1)):
                for j0 in (0, NCH):
                    pt = ps.tile([128, NCH], f32, tag="ps")
                    nc.tensor.matmul(
                        pt[:, :], wb, hlT[:, j0 : j0 + NCH], start=True, stop=True
                    )
                    nc.vector.tensor_copy(tT[:, j0 : j0 + NCH], pt[:, :])

            # ---- out[iblk] = tTk[:, iblk].T @ hrT (+bias, k-interleaved)
            for iblk in range(8):
                obuf = ob.tile([128, 2 * N], f32)
                for j0 in (0, NCH):
                    for k, tT in ((0, tT0), (1, tT1)):
                        po = ps.tile([128, NCH], f32, tag="ps")
                        nc.tensor.matmul(
                            po[:, :],
                            tT[:, iblk * 128 : (iblk + 1) * 128],
                            hrT[:, j0 : j0 + NCH],
                            start=True, stop=True,
                        )
                        dst = obuf[:, 2 * j0 + k : 2 * j0 + k + 2 * NCH - k : 2]
                        if k == 0:
                            nc.vector.tensor_scalar_add(dst, po[:, :], bb_s[:, 0:1])
                        else:
                            nc.scalar.activation(
                                dst, po[:, :], AF.Identity,
                                bias=bb_s[:, 1:2], scale=1.0,
                            )
                nc.sync.dma_start(out=out_v[iblk], in_=obuf)

    nc.finalize()
    return nc


def kernel(x_l, W_l, b_l, W_r, b_r, W_bil, b_bil):
    from concourse import bass_utils

    if "nc" not in _cached:
        _cached["nc"] = _build()
    nc = _cached["nc"]

    com = {
        "w_l": np.ascontiguousarray(W_l, np.float32),
        "b_l": np.ascontiguousarray(b_l, np.float32),
        "w_r": np.ascontiguousarray(W_r, np.float32),
        "b_r": np.ascontiguousarray(b_r, np.float32),
        "w_bil": np.ascontiguousarray(W_bil, np.float32),
        "b_bil": np.ascontiguousarray(b_bil, np.float32),
    }
    in_maps = [
        {"x": np.ascontiguousarray(x_l[c], np.float32), **com} for c in range(B)
    ]
    res = bass_utils.run_bass_kernel_spmd(nc, in_maps, core_ids=list(range(B)))
    return np.stack([res.results[c]["out"] for c in range(B)], axis=0)


# revision 4
# speedup vs baseline: 1.1964x; 1.1964x over previous
"""Bass/Trainium2 kernel for nn_BilinearPairedLayer.

Math (per batch b):
  xl = concat([x, shift_down(x,1), shift_up(x,1)], -1)      # [N, 192]
  xr = concat([x, shift_up(x,1), shift_down(x,1)], -1)
  hl = relu(xl @ W_l.T + b_l)                               # [N, 128]
  hr = relu(xr @ W_r.T + b_r)
  out[i,j,k] = sum_g (hl @ W_bil[k])[i,g] * hr[j,g] + b_bil[k]   # [N, N, 2]

Sharding: data-parallel over B — core c computes batch b=c (B=8, 8 cores).

Per-core dataflow (all fp32):
  - PE warmup spinner trips the HAM clock gate (1.2 -> 2.4 GHz) while the
    input DMAs land, so the real matmuls run warm
  - transpose x -> xT [64, N] via PE transposes (8 tiles of [128, 64]);
    the context shifts are then free: shifted feature chunks of xl^T are
    column-offset views of xT, so hl^T accumulates 3 matmuls per N-chunk
    with restricted column ranges at the sequence edges
  - hlT/hrT [128, N] = relu(W @ xlT + b) with per-partition ACT bias
  - tTk [128, N] = W_bil[k] (stationary, as stored) @ hlT
  - out tile [i=128, j=512] = tTk[:, iblk].T @ hrT chunk  (PSUM, contiguous)
  - bias + (j,k)-interleave fused into the mandatory PSUM->SBUF copy:
    DVE tensor_scalar_add writes k=0 stride-2 columns, ACT activation
    (Identity, per-partition bias) writes k=1 — then a 512 KiB DMA per
    half row-block. Emission is phased so the first output DMA can start
    after only the j<512 dependency chain.
"""

import numpy as np

B, N, NIN = 8, 1024, 64
H = 128
NOUT = 2
NCH = 512  # matmul free-dim chunk (one PSUM bank of fp32)
NWARM = 20

_cached = {}


def _build():
    import concourse.bacc as bacc
    import concourse.mybir as mybir
    import concourse.tile as tile
    from concourse.masks import make_identity

    f32 = mybir.dt.float32
    AF = mybir.ActivationFunctionType

    nc = bacc.Bacc("TRN2", target_bir_lowering=False, debug=False, num_devices=8)

    x_d = nc.dram_tensor("x", [N, NIN], f32, kind="ExternalInput").ap()
    wl_d = nc.dram_tensor("w_l", [H, 3 * NIN], f32, kind="ExternalInput").ap()
    bl_d = nc.dram_tensor("b_l", [H], f32, kind="ExternalInput").ap()
    wr_d = nc.dram_tensor("w_r", [H, 3 * NIN], f32, kind="ExternalInput").ap()
    br_d = nc.dram_tensor("b_r", [H], f32, kind="ExternalInput").ap()
    wb_d = nc.dram_tensor("w_bil", [NOUT, H, H], f32, kind="ExternalInput").ap()
    bb_d = nc.dram_tensor("b_bil", [NOUT], f32, kind="ExternalInput").ap()
    out_d = nc.dram_tensor("out", [N, N, NOUT], f32, kind="ExternalOutput").ap()
    # row-block view: [8 blocks, 128 rows, (j,k) interleaved 2048]
    out_v = out_d.rearrange("(t p) n k -> t p (n k)", p=128)

    with tile.TileContext(nc) as tc:
        with (
            tc.tile_pool(name="const", bufs=1) as const,
            tc.tile_pool(name="ps", bufs=7, space="PSUM") as ps,
            tc.tile_pool(name="wps", bufs=1, space="PSUM") as wps_pool,
            tc.tile_pool(name="ob", bufs=4) as ob,
        ):
            # ---- input DMAs, most critical first
            xload = const.tile([128, 8, NIN], f32)
            nc.sync.dma_start(out=xload, in_=x_d.rearrange("(t p) f -> p t f", p=128))
            wl_s = const.tile([H, 3 * NIN], f32)
            nc.sync.dma_start(out=wl_s, in_=wl_d)
            wr_s = const.tile([H, 3 * NIN], f32)
            nc.sync.dma_start(out=wr_s, in_=wr_d)
            wb0 = const.tile([H, H], f32)
            nc.gpsimd.dma_start(out=wb0, in_=wb_d[0])
            wb1 = const.tile([H, H], f32)
            nc.gpsimd.dma_start(out=wb1, in_=wb_d[1])
            bl_s = const.tile([H, 1], f32)
            nc.gpsimd.dma_start(out=bl_s, in_=bl_d.unsqueeze(1))
            br_s = const.tile([H, 1], f32)
            nc.gpsimd.dma_start(out=br_s, in_=br_d.unsqueeze(1))
            bb_s = const.tile([128, NOUT], f32)
            nc.gpsimd.dma_start(
                out=bb_s, in_=bb_d.unsqueeze(0).broadcast_to([128, NOUT])
            )

            # ---- PE warmup spinner (trips HAM while DMAs land)
            warm = const.tile([128, 128], f32)
            nc.vector.memset(warm, 0.0)
            wps = wps_pool.tile([128, NCH], f32, tag="warm")
            for _ in range(NWARM):
                nc.tensor.matmul(
                    wps[:, 0:128], warm, warm, start=True, stop=True,
                    skip_group_check=True,
                )

            ident = const.tile([128, 128], f32)
            make_identity(nc, ident)

            # ---- transpose W chunks -> lhsT layout [64, 128] each
            wlT = const.tile([NIN, 3, H], f32)
            wrT = const.tile([NIN, 3, H], f32)
            for wt, ws in ((wlT, wl_s), (wrT, wr_s)):
                for c in range(3):
                    pst = ps.tile([128, NCH], f32, tag="ps")
                    nc.tensor.transpose(
                        pst[0:NIN, 0:H], ws[:, c * NIN : (c + 1) * NIN], ident
                    )
                    nc.scalar.copy(wt[:, c, :], pst[0:NIN, 0:H])

            # ---- transpose x -> xT [64, 1024]
            xT = const.tile([NIN, N], f32)
            for t in range(8):
                pst = ps.tile([128, NCH], f32, tag="ps")
                nc.tensor.transpose(pst[0:NIN, 0:128], xload[:, t, :], ident)
                nc.scalar.copy(xT[:, t * 128 : (t + 1) * 128], pst[0:NIN, 0:128])

            hlT = const.tile([H, N], f32)
            hrT = const.tile([H, N], f32)

            def h_chunk(dst, wt, bias, s1, j0):
                # chunk 1 is shift_down (src col i-1) for xl, shift_up (i+1) for xr
                ph = ps.tile([128, NCH], f32, tag="ps")
                nc.tensor.matmul(
                    ph[:, :], wt[:, 0, :], xT[:, j0 : j0 + NCH],
                    start=True, stop=False,
                )
                for c, s in ((1, s1), (2, -s1)):
                    lo = max(j0, -s)           # first valid out col
                    hi = min(j0 + NCH, N - s)  # one past last valid out col
                    nc.tensor.matmul(
                        ph[:, lo - j0 : hi - j0],
                        wt[:, c, :],
                        xT[:, lo + s : hi + s],
                        start=False, stop=(c == 2),
                    )
                nc.scalar.activation(
                    dst[:, j0 : j0 + NCH], ph[:, :], AF.Relu,
                    bias=bias[:, 0:1], scale=1.0,
                )

            tT0 = const.tile([H, N], f32)
            tT1 = const.tile([H, N], f32)

            def t_chunk(wb, tT, j0):
                pt = ps.tile([128, NCH], f32, tag="ps")
                nc.tensor.matmul(
                    pt[:, :], wb, hlT[:, j0 : j0 + NCH], start=True, stop=True
                )
                nc.vector.tensor_copy(tT[:, j0 : j0 + NCH], pt[:, :])

            def out_half(iblk, j0):
                ohalf = ob.tile([128, 2 * NCH], f32, tag="ob")
                for k, tT in ((0, tT0), (1, tT1)):
                    po = ps.tile([128, NCH], f32, tag="ps")
                    nc.tensor.matmul(
                        po[:, :],
                        tT[:, iblk * 128 : (iblk + 1) * 128],
                        hrT[:, j0 : j0 + NCH],
                        start=True, stop=True,
                    )
                    dst = ohalf[:, k : 2 * NCH : 2]
                    if k == 0:
                        nc.vector.tensor_scalar_add(dst, po[:, :], bb_s[:, 0:1])
                    else:
                        nc.scalar.activation(
                            dst, po[:, :], AF.Identity, bias=bb_s[:, 1:2], scale=1.0
                        )
                nc.sync.dma_start(
                    out=out_v[iblk][:, 2 * j0 : 2 * j0 + 2 * NCH], in_=ohalf
                )

            # phased emission: everything needed for j<512 outputs first
            h_chunk(hlT, wlT, bl_s, -1, 0)
            h_chunk(hrT, wrT, br_s, +1, 0)
            t_chunk(wb0, tT0, 0)
            t_chunk(wb1, tT1, 0)
            for iblk in range(4):
                out_half(iblk, 0)
            h_chunk(hlT, wlT, bl_s, -1, NCH)
            h_chunk(hrT, wrT, br_s, +1, NCH)
            t_chunk(wb0, tT0, NCH)
            t_chunk(wb1, tT1, NCH)
            for iblk in range(4):
                out_half(iblk, NCH)
            for iblk in range(4, 8):
                out_half(iblk, 0)
                out_half(iblk, NCH)

    nc.finalize()
    return nc


def kernel(x_l, W_l, b_l, W_r, b_r, W_bil, b_bil):
    from concourse import bass_utils

    if "nc" not in _cached:
        _cached["nc"] = _build()
    nc = _cached["nc"]

    com = {
        "w_l": np.ascontiguousarray(W_l, np.float32),
        "b_l": np.ascontiguousarray(b_l, np.float32),
        "w_r": np.ascontiguousarray(W_r, np.float32),
        "b_r": np.ascontiguousarray(b_r, np.float32),
        "w_bil": np.ascontiguousarray(W_bil, np.float32),
        "b_bil": np.ascontiguousarray(b_bil, np.float32),
    }
    in_maps = [
        {"x": np.ascontiguousarray(x_l[c], np.float32), **com} for c in range(B)
    ]
    res = bass_utils.run_bass_kernel_spmd(nc, in_maps, core_ids=list(range(B)))
    return np.stack([res.results[c]["out"] for c in range(B)], axis=0)


# revision 8
# speedup vs baseline: 1.5418x; 1.2887x over previous
"""Bass/Trainium2 kernel for nn_BilinearPairedLayer.

Math (per batch b):
  xl = concat([x, shift_down(x,1), shift_up(x,1)], -1)      # [N, 192]
  xr = concat([x, shift_up(x,1), shift_down(x,1)], -1)
  hl = relu(xl @ W_l.T + b_l)                               # [N, 128]
  hr = relu(xr @ W_r.T + b_r)
  out[i,j,k] = sum_g (hl @ W_bil[k])[i,g] * hr[j,g] + b_bil[k]   # [N, N, 2]

Sharding: data-parallel over B — core c computes batch b=c (B=8, 8 cores).

Per-core dataflow:
  - all matmuls run in float32r (the TRN2 fp32 fast-matmul format, ~12-bit
    mantissa, 1 cycle/row vs 4 for plain fp32); rounding to f32r is fused
    into the PSUM->SBUF copies that have to happen anyway
  - PE warmup spinner trips the HAM clock gate (1.2 -> 2.4 GHz) while the
    input DMAs land
  - transpose x -> xT [64, N] via PE transposes; xT carries one zero guard
    column on each side so the context-shift matmuls are always full
    512-column accumulations (shifted chunks of xl^T are column-offset
    views of xT)
  - hlT/hrT [128, N] = relu(W @ xlT + b) with per-partition ACT bias
  - tTk [128, N] = W_bil[k] (stationary, as stored) @ hlT
  - out tile [i=128, j=512] = tTk[:, iblk].T @ hrT chunk  (PSUM fp32)
  - b_bil + (j,k)-interleave fused into the PSUM->SBUF copy: DVE
    tensor_scalar_add writes k=0 stride-2 columns, ACT activation
    (Identity, per-partition bias) writes k=1 — then a 512 KiB DMA per
    half row-block, phased so the first output DMA starts early.
"""

import numpy as np

B, N, NIN = 8, 1024, 64
H = 128
NOUT = 2
NCH = 512  # matmul free-dim chunk (one PSUM bank of fp32)
NWARM = 8

_cached = {}


def _build():
    import concourse.bacc as bacc
    import concourse.mybir as mybir
    import concourse.tile as tile
    from concourse.masks import make_identity

    f32 = mybir.dt.float32
    f32r = mybir.dt.float32r
    AF = mybir.ActivationFunctionType

    nc = bacc.Bacc("TRN2", target_bir_lowering=False, debug=False, num_devices=8)

    x_d = nc.dram_tensor("x", [N, NIN], f32, kind="ExternalInput").ap()
    wl_d = nc.dram_tensor("w_l", [H, 3 * NIN], f32, kind="ExternalInput").ap()
    bl_d = nc.dram_tensor("b_l", [H], f32, kind="ExternalInput").ap()
    wr_d = nc.dram_tensor("w_r", [H, 3 * NIN], f32, kind="ExternalInput").ap()
    br_d = nc.dram_tensor("b_r", [H], f32, kind="ExternalInput").ap()
    wb_d = nc.dram_tensor("w_bil", [NOUT, H, H], f32, kind="ExternalInput").ap()
    bb_d = nc.dram_tensor("b_bil", [NOUT], f32, kind="ExternalInput").ap()
    out_d = nc.dram_tensor("out", [N, N, NOUT], f32, kind="ExternalOutput").ap()
    # row-block view: [8 blocks, 128 rows, (j,k) interleaved 2048]
    out_v = out_d.rearrange("(t p) n k -> t p (n k)", p=128)

    with tile.TileContext(nc) as tc:
        with (
            tc.tile_pool(name="const", bufs=1) as const,
            tc.tile_pool(name="ps", bufs=7, space="PSUM") as ps,
            tc.tile_pool(name="wps", bufs=1, space="PSUM") as wps_pool,
            tc.tile_pool(name="ob", bufs=4) as ob,
        ):
            # ---- input DMAs, most critical first
            xload = const.tile([128, 8, NIN], f32)
            nc.sync.dma_start(out=xload, in_=x_d.rearrange("(t p) f -> p t f", p=128))
            wl_s = const.tile([H, 3 * NIN], f32)
            nc.sync.dma_start(out=wl_s, in_=wl_d)
            wr_s = const.tile([H, 3 * NIN], f32)
            nc.sync.dma_start(out=wr_s, in_=wr_d)
            wb0_f = const.tile([H, H], f32)
            nc.gpsimd.dma_start(out=wb0_f, in_=wb_d[0])
            wb1_f = const.tile([H, H], f32)
            nc.gpsimd.dma_start(out=wb1_f, in_=wb_d[1])
            bl_s = const.tile([H, 1], f32)
            nc.gpsimd.dma_start(out=bl_s, in_=bl_d.unsqueeze(1))
            br_s = const.tile([H, 1], f32)
            nc.gpsimd.dma_start(out=br_s, in_=br_d.unsqueeze(1))
            bb_s = const.tile([128, NOUT], f32)
            nc.gpsimd.dma_start(
                out=bb_s, in_=bb_d.unsqueeze(0).broadcast_to([128, NOUT])
            )

            # ---- PE warmup spinner (trips HAM while DMAs land)
            warm = const.tile([128, 256], f32)
            nc.vector.memset(warm, 0.0)
            wps = wps_pool.tile([128, NCH], f32, tag="warm")
            for _ in range(NWARM):
                nc.tensor.matmul(
                    wps[:, 0:256], warm[:, 0:128], warm,
                    start=True, stop=True, skip_group_check=True,
                )

            ident = const.tile([128, 128], f32)
            make_identity(nc, ident)

            # f32r copies of the bilinear weights (lhsT = W_bil[k] as stored)
            wb0 = const.tile([H, H], f32r)
            nc.vector.tensor_copy(wb0[:], wb0_f[:])
            wb1 = const.tile([H, H], f32r)
            nc.vector.tensor_copy(wb1[:], wb1_f[:])

            # ---- transpose W chunks -> lhsT layout [64, 128] each
            wlT = const.tile([NIN, 3, H], f32r)
            wrT = const.tile([NIN, 3, H], f32r)
            for wt, ws in ((wlT, wl_s), (wrT, wr_s)):
                for c in range(3):
                    pst = ps.tile([128, NCH], f32, tag="ps")
                    nc.tensor.transpose(
                        pst[0:NIN, 0:H], ws[:, c * NIN : (c + 1) * NIN], ident
                    )
                    nc.scalar.copy(wt[:, c, :], pst[0:NIN, 0:H])

            # ---- transpose x -> xT [64, 2+1024+2] with zero guard columns
            GD = 2
            xT = const.tile([NIN, N + 2 * GD], f32r)
            nc.scalar.copy(xT[:, 0:GD], warm[0:NIN, 0:GD])
            nc.scalar.copy(xT[:, N + GD : N + 2 * GD], warm[0:NIN, 0:GD])
            for t in range(8):
                pst = ps.tile([128, NCH], f32, tag="ps")
                nc.tensor.transpose(pst[0:NIN, 0:128], xload[:, t, :], ident)
                nc.scalar.copy(
                    xT[:, GD + t * 128 : GD + (t + 1) * 128], pst[0:NIN, 0:128]
                )

            hlT = const.tile([H, N], f32r)
            hrT = const.tile([H, N], f32r)

            def h_chunk(dst, wt, bias, s1, j0):
                # chunk 1 is shift_down (src col i-1) for xl, shift_up (i+1) for xr
                ph = ps.tile([128, NCH], f32, tag="ps")
                for c, s in ((0, 0), (1, s1), (2, -s1)):
                    nc.tensor.matmul(
                        ph[:, :],
                        wt[:, c, :],
                        xT[:, GD + j0 + s : GD + j0 + s + NCH],
                        start=(c == 0), stop=(c == 2),
                    )
                nc.scalar.activation(
                    dst[:, j0 : j0 + NCH], ph[:, :], AF.Relu,
                    bias=bias[:, 0:1], scale=1.0,
                )

            tT0 = const.tile([H, N], f32r)
            tT1 = const.tile([H, N], f32r)

            def t_chunk(wb, tT, j0):
                pt = ps.tile([128, NCH], f32, tag="ps")
                nc.tensor.matmul(
                    pt[:, :], wb, hlT[:, j0 : j0 + NCH], start=True, stop=True
                )
                nc.vector.tensor_copy(tT[:, j0 : j0 + NCH], pt[:, :])

            def out_half(iblk, j0):
                ohalf = ob.tile([128, 2 * NCH], f32, tag="ob")
                for k, tT in ((0, tT0), (1, tT1)):
                    po = ps.tile([128, NCH], f32, tag="ps")
                    nc.tensor.matmul(
                        po[:, :],
                        tT[:, iblk * 128 : (iblk + 1) * 128],
                        hrT[:, j0 : j0 + NCH],
                        start=True, stop=True,
                    )
                    dst = ohalf[:, k : 2 * NCH : 2]
                    if k == 0:
                        nc.vector.tensor_scalar_add(dst, po[:, :], bb_s[:, 0:1])
                    else:
                        nc.scalar.activation(
                            dst, po[:, :], AF.Identity, bias=bb_s[:, 1:2], scale=1.0
                        )
                nc.sync.dma_start(
                    out=out_v[iblk][:, 2 * j0 : 2 * j0 + 2 * NCH], in_=ohalf
                )

            # phased emission: everything needed for j<512 outputs first
            h_chunk(hlT, wlT, bl_s, -1, 0)
            h_chunk(hrT, wrT, br_s, +1, 0)
            t_chunk(wb0, tT0, 0)
            t_chunk(wb1, tT1, 0)
            for iblk in range(4):
                out_half(iblk, 0)
            h_chunk(hlT, wlT, bl_s, -1, NCH)
            h_chunk(hrT, wrT, br_s, +1, NCH)
            t_chunk(wb0, tT0, NCH)
            t_chunk(wb1, tT1, NCH)
            for iblk in range(4):
                out_half(iblk, NCH)
            for iblk in range(4, 8):
                out_half(iblk, 0)
                out_half(iblk, NCH)

    nc.finalize()
    return nc


def kernel(x_l, W_l, b_l, W_r, b_r, W_bil, b_bil):
    from concourse import bass_utils

    if "nc" not in _cached:
        _cached["nc"] = _build()
    nc = _cached["nc"]

    com = {
        "w_l": np.ascontiguousarray(W_l, np.float32),
        "b_l": np.ascontiguousarray(b_l, np.float32),
        "w_r": np.ascontiguousarray(W_r, np.float32),
        "b_r": np.ascontiguousarray(b_r, np.float32),
        "w_bil": np.ascontiguousarray(W_bil, np.float32),
        "b_bil": np.ascontiguousarray(b_bil, np.float32),
    }
    in_maps = [
        {"x": np.ascontiguousarray(x_l[c], np.float32), **com} for c in range(B)
    ]
    res = bass_utils.run_bass_kernel_spmd(nc, in_maps, core_ids=list(range(B)))
    return np.stack([res.results[c]["out"] for c in range(B)], axis=0)


# revision 10
# speedup vs baseline: 1.6542x; 1.0729x over previous
"""Bass/Trainium2 kernel for nn_BilinearPairedLayer.

Math (per batch b):
  xl = concat([x, shift_down(x,1), shift_up(x,1)], -1)      # [N, 192]
  xr = concat([x, shift_up(x,1), shift_down(x,1)], -1)
  hl = relu(xl @ W_l.T + b_l)                               # [N, 128]
  hr = relu(xr @ W_r.T + b_r)
  out[i,j,k] = sum_g (hl @ W_bil[k])[i,g] * hr[j,g] + b_bil[k]   # [N, N, 2]

Sharding: data-parallel over B — core c computes batch b=c (B=8, 8 cores).
The host-side shard step also re-lays-out the inputs: x arrives transposed
with zero guard columns ([64, 2+1024+2]) and W_l/W_r arrive as per-chunk
lhsT tiles [64, 3, 128], so the device never transposes anything.

Per-core dataflow:
  - all matmuls run in float32r (the TRN2 fp32 fast-matmul format, ~12-bit
    mantissa, ~2x faster than plain fp32); the f32r rounding happens in
    cheap on-chip copy passes (DVE) right after the input DMAs land
  - a short PE warmup spinner fills the pre-input window and helps trip
    the HAM clock gate (1.2 -> 2.4 GHz) early
  - the context shifts are free: shifted feature chunks of xl^T are
    column-offset views of xT thanks to the guard columns, so hlT/hrT
    accumulate 3 full 512-column matmuls per N-chunk
  - hlT/hrT [128, N] = relu(W @ xlT + b) with per-partition ACT bias
  - tTk [128, N] = W_bil[k] (stationary, as stored) @ hlT
  - out tile [i=128, j=512] = tTk[:, iblk].T @ hrT chunk  (PSUM fp32)
  - b_bil + (j,k)-interleave fused into the PSUM->SBUF copy: DVE
    tensor_scalar_add writes k=0 stride-2 columns, ACT activation
    (Identity, per-partition bias) writes k=1 — then a 512 KiB DMA per
    half row-block; emission ordered so the first DMA starts early and
    the PE never idles long enough to re-throttle.
"""

import numpy as np

B, N, NIN = 8, 1024, 64
H = 128
NOUT = 2
NCH = 512  # matmul free-dim chunk (one PSUM bank of fp32)
GD = 2     # zero guard columns on each side of xT
NWARM = 3

_cached = {}


def _build():
    import concourse.bacc as bacc
    import concourse.mybir as mybir
    import concourse.tile as tile

    f32 = mybir.dt.float32
    f32r = mybir.dt.float32r
    AF = mybir.ActivationFunctionType

    nc = bacc.Bacc("TRN2", target_bir_lowering=False, debug=False, num_devices=8)

    xt_d = nc.dram_tensor("x_t", [NIN, N + 2 * GD], f32, kind="ExternalInput").ap()
    wlt_d = nc.dram_tensor("w_lt", [NIN, 3, H], f32, kind="ExternalInput").ap()
    bl_d = nc.dram_tensor("b_l", [H], f32, kind="ExternalInput").ap()
    wrt_d = nc.dram_tensor("w_rt", [NIN, 3, H], f32, kind="ExternalInput").ap()
    br_d = nc.dram_tensor("b_r", [H], f32, kind="ExternalInput").ap()
    wb_d = nc.dram_tensor("w_bil", [NOUT, H, H], f32, kind="ExternalInput").ap()
    bb_d = nc.dram_tensor("b_bil", [NOUT], f32, kind="ExternalInput").ap()
    out_d = nc.dram_tensor("out", [N, N, NOUT], f32, kind="ExternalOutput").ap()
    # row-block view: [8 blocks, 128 rows, (j,k) interleaved 2048]
    out_v = out_d.rearrange("(t p) n k -> t p (n k)", p=128)

    with tile.TileContext(nc) as tc:
        with (
            tc.tile_pool(name="const", bufs=1) as const,
            tc.tile_pool(name="ps", bufs=7, space="PSUM") as ps,
            tc.tile_pool(name="wps", bufs=1, space="PSUM") as wps_pool,
            tc.tile_pool(name="ob", bufs=4) as ob,
        ):
            # ---- input DMAs, most critical first
            xT_f = const.tile([NIN, N + 2 * GD], f32)
            nc.sync.dma_start(out=xT_f, in_=xt_d)
            wlT_f = const.tile([NIN, 3, H], f32)
            nc.sync.dma_start(out=wlT_f, in_=wlt_d)
            wrT_f = const.tile([NIN, 3, H], f32)
            nc.sync.dma_start(out=wrT_f, in_=wrt_d)
            wb0_f = const.tile([H, H], f32)
            nc.gpsimd.dma_start(out=wb0_f, in_=wb_d[0])
            wb1_f = const.tile([H, H], f32)
            nc.gpsimd.dma_start(out=wb1_f, in_=wb_d[1])
            bl_s = const.tile([H, 1], f32)
            nc.gpsimd.dma_start(out=bl_s, in_=bl_d.unsqueeze(1))
            br_s = const.tile([H, 1], f32)
            nc.gpsimd.dma_start(out=br_s, in_=br_d.unsqueeze(1))
            bb_s = const.tile([128, NOUT], f32)
            nc.gpsimd.dma_start(
                out=bb_s, in_=bb_d.unsqueeze(0).broadcast_to([128, NOUT])
            )

            # ---- PE warmup spinner (fills the pre-input idle window)
            warm = const.tile([128, 256], f32)
            nc.vector.memset(warm, 0.0)
            wps = wps_pool.tile([128, NCH], f32, tag="warm")
            for _ in range(NWARM):
                nc.tensor.matmul(
                    wps[:, 0:256], warm[:, 0:128], warm,
                    start=True, stop=True, skip_group_check=True,
                )

            # ---- f32r rounding copies (DVE)
            xT = const.tile([NIN, N + 2 * GD], f32r)
            nc.vector.tensor_copy(xT[:], xT_f[:])
            wlT = const.tile([NIN, 3, H], f32r)
            nc.vector.tensor_copy(wlT[:], wlT_f[:])
            wrT = const.tile([NIN, 3, H], f32r)
            nc.vector.tensor_copy(wrT[:], wrT_f[:])
            wb0 = const.tile([H, H], f32r)
            nc.vector.tensor_copy(wb0[:], wb0_f[:])
            wb1 = const.tile([H, H], f32r)
            nc.vector.tensor_copy(wb1[:], wb1_f[:])

            hlT = const.tile([H, N], f32r)
            hrT = const.tile([H, N], f32r)

            def h_chunk(dst, wt, bias, s1, j0):
                # chunk 1 is shift_down (src col i-1) for xl, shift_up (i+1) for xr
                ph = ps.tile([128, NCH], f32, tag="ps")
                for c, s in ((0, 0), (1, s1), (2, -s1)):
                    nc.tensor.matmul(
                        ph[:, :],
                        wt[:, c, :],
                        xT[:, GD + j0 + s : GD + j0 + s + NCH],
                        start=(c == 0), stop=(c == 2),
                    )
                nc.scalar.activation(
                    dst[:, j0 : j0 + NCH], ph[:, :], AF.Relu,
                    bias=bias[:, 0:1], scale=1.0,
                )

            tT0 = const.tile([H, N], f32r)
            tT1 = const.tile([H, N], f32r)

            def t_chunk(wb, tT, j0):
                pt = ps.tile([128, NCH], f32, tag="ps")
                nc.tensor.matmul(
                    pt[:, :], wb, hlT[:, j0 : j0 + NCH], start=True, stop=True
                )
                nc.vector.tensor_copy(tT[:, j0 : j0 + NCH], pt[:, :])

            def out_half(iblk, j0):
                ohalf = ob.tile([128, 2 * NCH], f32, tag="ob")
                for k, tT in ((0, tT0), (1, tT1)):
                    po = ps.tile([128, NCH], f32, tag="ps")
                    nc.tensor.matmul(
                        po[:, :],
                        tT[:, iblk * 128 : (iblk + 1) * 128],
                        hrT[:, j0 : j0 + NCH],
                        start=True, stop=True,
                    )
                    dst = ohalf[:, k : 2 * NCH : 2]
                    if k == 0:
                        nc.vector.tensor_scalar_add(dst, po[:, :], bb_s[:, 0:1])
                    else:
                        nc.scalar.activation(
                            dst, po[:, :], AF.Identity, bias=bb_s[:, 1:2], scale=1.0
                        )
                nc.sync.dma_start(
                    out=out_v[iblk][:, 2 * j0 : 2 * j0 + 2 * NCH], in_=ohalf
                )

            # emission order: earliest first output DMA, PE kept dense
            h_chunk(hlT, wlT, bl_s, -1, 0)
            h_chunk(hrT, wrT, br_s, +1, 0)
            h_chunk(hlT, wlT, bl_s, -1, NCH)   # PE filler while relu0 lands
            h_chunk(hrT, wrT, br_s, +1, NCH)
            t_chunk(wb0, tT0, 0)
            t_chunk(wb1, tT1, 0)
            for iblk in range(4):
                out_half(iblk, 0)
            t_chunk(wb0, tT0, NCH)
            t_chunk(wb1, tT1, NCH)
            for iblk in range(4):
                out_half(iblk, NCH)
            for iblk in range(4, 8):
                out_half(iblk, 0)
                out_half(iblk, NCH)

    nc.finalize()
    return nc


def make_in_maps(x_l, W_l, b_l, W_r, b_r, W_bil, b_bil):
    # host-side layout: W chunks to lhsT [f=64, chunk, h], x to [64, N] with
    # zero guard columns
    def w_chunks(W):
        return np.ascontiguousarray(
            np.asarray(W, np.float32).reshape(H, 3, NIN).transpose(2, 1, 0)
        )

    x_l = np.asarray(x_l, np.float32)
    xt = np.zeros((B, NIN, N + 2 * GD), np.float32)
    xt[:, :, GD : GD + N] = x_l.transpose(0, 2, 1)

    com = {
        "w_lt": w_chunks(W_l),
        "b_l": np.ascontiguousarray(b_l, np.float32),
        "w_rt": w_chunks(W_r),
        "b_r": np.ascontiguousarray(b_r, np.float32),
        "w_bil": np.ascontiguousarray(W_bil, np.float32),
        "b_bil": np.ascontiguousarray(b_bil, np.float32),
    }
    return [{"x_t": np.ascontiguousarray(xt[c]), **com} for c in range(B)]


def kernel(x_l, W_l, b_l, W_r, b_r, W_bil, b_bil):
    from concourse import bass_utils

    if "nc" not in _cached:
        _cached["nc"] = _build()
    nc = _cached["nc"]

    in_maps = make_in_maps(x_l, W_l, b_l, W_r, b_r, W_bil, b_bil)
    res = bass_utils.run_bass_kernel_spmd(nc, in_maps, core_ids=list(range(B)))
    return np.stack([res.results[c]["out"] for c in range(B)], axis=0)


# revision 11
# speedup vs baseline: 1.7238x; 1.0421x over previous
"""Bass/Trainium2 kernel for nn_BilinearPairedLayer.

Math (per batch b):
  xl = concat([x, shift_down(x,1), shift_up(x,1)], -1)      # [N, 192]
  xr = concat([x, shift_up(x,1), shift_down(x,1)], -1)
  hl = relu(xl @ W_l.T + b_l)                               # [N, 128]
  hr = relu(xr @ W_r.T + b_r)
  out[i,j,k] = sum_g (hl @ W_bil[k])[i,g] * hr[j,g] + b_bil[k]   # [N, N, 2]

Sharding: data-parallel over B — core c computes batch b=c (B=8, 8 cores).
The host-side shard step also re-lays-out the inputs: x arrives transposed
with zero guard columns ([64, 2+1024+2]) and W_l/W_r arrive as per-chunk
lhsT tiles [64, 3, 128], so the device never transposes anything.

Per-core dataflow:
  - all matmuls run in float32r (the TRN2 fp32 fast-matmul format, ~12-bit
    mantissa, ~2x faster than plain fp32); the f32r rounding happens in
    cheap on-chip copy passes (DVE) right after the input DMAs land
  - a short PE warmup spinner fills the pre-input window and helps trip
    the HAM clock gate (1.2 -> 2.4 GHz) early
  - the context shifts are free: shifted feature chunks of xl^T are
    column-offset views of xT thanks to the guard columns, so hlT/hrT
    accumulate 3 full 512-column matmuls per N-chunk
  - hlT/hrT [128, N] = relu(W @ xlT + b) with per-partition ACT bias
  - tTk [128, N] = W_bil[k] (stationary, as stored) @ hlT
  - out tile [i=128, j=512] = tTk[:, iblk].T @ hrT chunk  (PSUM fp32)
  - b_bil + (j,k)-interleave fused into the PSUM->SBUF copy: DVE
    tensor_scalar_add writes k=0 stride-2 columns, ACT activation
    (Identity, per-partition bias) writes k=1 — then a 512 KiB DMA per
    half row-block; emission ordered so the first DMA starts early and
    the PE never idles long enough to re-throttle.
"""

import numpy as np

B, N, NIN = 8, 1024, 64
H = 128
NOUT = 2
NCH = 512  # matmul free-dim chunk (one PSUM bank of fp32)
GD = 2     # zero guard columns on each side of xT
NWARM = 3

_cached = {}


def _build():
    import concourse.bacc as bacc
    import concourse.mybir as mybir
    import concourse.tile as tile

    f32 = mybir.dt.float32
    f32r = mybir.dt.float32r
    AF = mybir.ActivationFunctionType

    nc = bacc.Bacc("TRN2", target_bir_lowering=False, debug=False, num_devices=8)

    xt_d = nc.dram_tensor("x_t", [NIN, N + 2 * GD], f32, kind="ExternalInput").ap()
    wlt_d = nc.dram_tensor("w_lt", [NIN, 3, H], f32, kind="ExternalInput").ap()
    bl_d = nc.dram_tensor("b_l", [H], f32, kind="ExternalInput").ap()
    wrt_d = nc.dram_tensor("w_rt", [NIN, 3, H], f32, kind="ExternalInput").ap()
    br_d = nc.dram_tensor("b_r", [H], f32, kind="ExternalInput").ap()
    wb_d = nc.dram_tensor("w_bil", [NOUT, H, H], f32, kind="ExternalInput").ap()
    bb_d = nc.dram_tensor("b_bil", [NOUT], f32, kind="ExternalInput").ap()
    out_d = nc.dram_tensor("out", [N, N, NOUT], f32, kind="ExternalOutput").ap()
    # row-block view: [8 blocks, 128 rows, (j,k) interleaved 2048]
    out_v = out_d.rearrange("(t p) n k -> t p (n k)", p=128)

    with tile.TileContext(nc) as tc:
        with (
            tc.tile_pool(name="const", bufs=1) as const,
            tc.tile_pool(name="ps", bufs=7, space="PSUM") as ps,
            tc.tile_pool(name="wps", bufs=1, space="PSUM") as wps_pool,
            tc.tile_pool(name="ob", bufs=4) as ob,
        ):
            # ---- input DMAs, most critical first
            xT_f = const.tile([NIN, N + 2 * GD], f32)
            nc.sync.dma_start(out=xT_f, in_=xt_d)
            wlT_f = const.tile([NIN, 3, H], f32)
            nc.sync.dma_start(out=wlT_f, in_=wlt_d)
            wrT_f = const.tile([NIN, 3, H], f32)
            nc.sync.dma_start(out=wrT_f, in_=wrt_d)

            # ---- PE warmup spinner (fills the pre-input idle window)
            warm = const.tile([128, 256], f32)
            nc.vector.memset(warm, 0.0)
            # dummy ACT ops: pull the lazy ACT table load to the front
            nc.scalar.activation(warm[0:1, 0:2], warm[0:1, 0:2], AF.Relu)
            nc.scalar.activation(warm[0:1, 2:4], warm[0:1, 0:2], AF.Identity)
            wps = wps_pool.tile([128, NCH], f32, tag="warm")
            for _ in range(NWARM):
                nc.tensor.matmul(
                    wps[:, 0:256], warm[:, 0:128], warm,
                    start=True, stop=True, skip_group_check=True,
                )

            # ---- f32r rounding copies on DVE, critical (x) first
            xT = const.tile([NIN, N + 2 * GD], f32r)
            nc.vector.tensor_copy(xT[:], xT_f[:])
            wlT = const.tile([NIN, 3, H], f32r)
            nc.vector.tensor_copy(wlT[:], wlT_f[:])
            wrT = const.tile([NIN, 3, H], f32r)
            nc.vector.tensor_copy(wrT[:], wrT_f[:])

            # ---- non-critical loads (SWDGE), after the casts in program order
            bl_s = const.tile([H, 1], f32)
            nc.gpsimd.dma_start(out=bl_s, in_=bl_d.unsqueeze(1))
            br_s = const.tile([H, 1], f32)
            nc.gpsimd.dma_start(out=br_s, in_=br_d.unsqueeze(1))
            bb_s = const.tile([128, NOUT], f32)
            nc.gpsimd.dma_start(
                out=bb_s, in_=bb_d.unsqueeze(0).broadcast_to([128, NOUT])
            )
            wb0_f = const.tile([H, H], f32)
            nc.gpsimd.dma_start(out=wb0_f, in_=wb_d[0])
            wb1_f = const.tile([H, H], f32)
            nc.gpsimd.dma_start(out=wb1_f, in_=wb_d[1])
            wb0 = const.tile([H, H], f32r)
            nc.scalar.copy(wb0[:], wb0_f[:])
            wb1 = const.tile([H, H], f32r)
            nc.scalar.copy(wb1[:], wb1_f[:])

            hlT = const.tile([H, N], f32r)
            hrT = const.tile([H, N], f32r)

            def h_chunk(dst, wt, bias, s1, j0):
                # chunk 1 is shift_down (src col i-1) for xl, shift_up (i+1) for xr
                ph = ps.tile([128, NCH], f32, tag="ps")
                for c, s in ((0, 0), (1, s1), (2, -s1)):
                    nc.tensor.matmul(
                        ph[:, :],
                        wt[:, c, :],
                        xT[:, GD + j0 + s : GD + j0 + s + NCH],
                        start=(c == 0), stop=(c == 2),
                    )
                nc.scalar.activation(
                    dst[:, j0 : j0 + NCH], ph[:, :], AF.Relu,
                    bias=bias[:, 0:1], scale=1.0,
                )

            tT0 = const.tile([H, N], f32r)
            tT1 = const.tile([H, N], f32r)

            def t_chunk(wb, tT, j0):
                pt = ps.tile([128, NCH], f32, tag="ps")
                nc.tensor.matmul(
                    pt[:, :], wb, hlT[:, j0 : j0 + NCH], start=True, stop=True
                )
                nc.vector.tensor_copy(tT[:, j0 : j0 + NCH], pt[:, :])

            def out_half(iblk, j0):
                ohalf = ob.tile([128, 2 * NCH], f32, tag="ob")
                for k, tT in ((0, tT0), (1, tT1)):
                    po = ps.tile([128, NCH], f32, tag="ps")
                    nc.tensor.matmul(
                        po[:, :],
                        tT[:, iblk * 128 : (iblk + 1) * 128],
                        hrT[:, j0 : j0 + NCH],
                        start=True, stop=True,
                    )
                    dst = ohalf[:, k : 2 * NCH : 2]
                    if k == 0:
                        nc.vector.tensor_scalar_add(dst, po[:, :], bb_s[:, 0:1])
                    else:
                        nc.scalar.activation(
                            dst, po[:, :], AF.Identity, bias=bb_s[:, 1:2], scale=1.0
                        )
                nc.sync.dma_start(
                    out=out_v[iblk][:, 2 * j0 : 2 * j0 + 2 * NCH], in_=ohalf
                )

            # emission order: earliest first output DMA, PE kept dense
            h_chunk(hlT, wlT, bl_s, -1, 0)
            h_chunk(hrT, wrT, br_s, +1, 0)
            h_chunk(hlT, wlT, bl_s, -1, NCH)   # PE filler while relu0 lands
            h_chunk(hrT, wrT, br_s, +1, NCH)
            t_chunk(wb0, tT0, 0)
            t_chunk(wb1, tT1, 0)
            for iblk in range(4):
                out_half(iblk, 0)
            t_chunk(wb0, tT0, NCH)
            t_chunk(wb1, tT1, NCH)
            for iblk in range(4):
                out_half(iblk, NCH)
            for iblk in range(4, 8):
                out_half(iblk, 0)
                out_half(iblk, NCH)

    nc.finalize()
    return nc


def make_in_maps(x_l, W_l, b_l, W_r, b_r, W_bil, b_bil):
    # host-side layout: W chunks to lhsT [f=64, chunk, h], x to [64, N] with
    # zero guard columns
    def w_chunks(W):
        return np.ascontiguousarray(
            np.asarray(W, np.float32).reshape(H, 3, NIN).transpose(2, 1, 0)
        )

    x_l = np.asarray(x_l, np.float32)
    xt = np.zeros((B, NIN, N + 2 * GD), np.float32)
    xt[:, :, GD : GD + N] = x_l.transpose(0, 2, 1)

    com = {
        "w_lt": w_chunks(W_l),
        "b_l": np.ascontiguousarray(b_l, np.float32),
        "w_rt": w_chunks(W_r),
        "b_r": np.ascontiguousarray(b_r, np.float32),
        "w_bil": np.ascontiguousarray(W_bil, np.float32),
        "b_bil": np.ascontiguousarray(b_bil, np.float32),
    }
    return [{"x_t": np.ascontiguousarray(xt[c]), **com} for c in range(B)]


def kernel(x_l, W_l, b_l, W_r, b_r, W_bil, b_bil):
    from concourse import bass_utils

    if "nc" not in _cached:
        _cached["nc"] = _build()
    nc = _cached["nc"]

    in_maps = make_in_maps(x_l, W_l, b_l, W_r, b_r, W_bil, b_bil)
    res = bass_utils.run_bass_kernel_spmd(nc, in_maps, core_ids=list(range(B)))
    return np.stack([res.results[c]["out"] for c in range(B)], axis=0)


# revision 13
# speedup vs baseline: 1.7484x; 1.0143x over previous
"""Bass/Trainium2 kernel for nn_BilinearPairedLayer.

Math (per batch b):
  xl = concat([x, shift_down(x,1), shift_up(x,1)], -1)      # [N, 192]
  xr = concat([x, shift_up(x,1), shift_down(x,1)], -1)
  hl = relu(xl @ W_l.T + b_l)                               # [N, 128]
  hr = relu(xr @ W_r.T + b_r)
  out[i,j,k] = sum_g (hl @ W_bil[k])[i,g] * hr[j,g] + b_bil[k]   # [N, N, 2]

Sharding: data-parallel over B — core c computes batch b=c (B=8, 8 cores).
The host-side shard step also re-lays-out the inputs: x arrives transposed
with zero guard columns ([64, 2+1024+2]) and W_l/W_r arrive as per-chunk
lhsT tiles [64, 3, 128], so the device never transposes anything.

Per-core dataflow:
  - all matmuls run in float32r (the TRN2 fp32 fast-matmul format, ~12-bit
    mantissa, ~2x faster than plain fp32); the f32r rounding happens in
    cheap on-chip copy passes (DVE) right after the input DMAs land
  - a short PE warmup spinner fills the pre-input window and helps trip
    the HAM clock gate (1.2 -> 2.4 GHz) early
  - the context shifts are free: shifted feature chunks of xl^T are
    column-offset views of xT thanks to the guard columns, so hlT/hrT
    accumulate 3 full 512-column matmuls per N-chunk
  - hlT/hrT [128, N] = relu(W @ xlT + b) with per-partition ACT bias
  - tTk [128, N] = W_bil[k] (stationary, as stored) @ hlT
  - out tile [i=128, j=512] = tTk[:, iblk].T @ hrT chunk  (PSUM fp32)
  - b_bil + (j,k)-interleave fused into the PSUM->SBUF copy: DVE
    tensor_scalar_add writes k=0 stride-2 columns, ACT activation
    (Identity, per-partition bias) writes k=1 — then a 512 KiB DMA per
    half row-block; emission ordered so the first DMA starts early and
    the PE never idles long enough to re-throttle.
"""

import numpy as np

B, N, NIN = 8, 1024, 64
H = 128
NOUT = 2
NCH = 512  # matmul free-dim chunk (one PSUM bank of fp32)
GD = 2     # zero guard columns on each side of xT
NWARM = 5

_cached = {}


def _build():
    import concourse.bacc as bacc
    import concourse.mybir as mybir
    import concourse.tile as tile

    f32 = mybir.dt.float32
    f32r = mybir.dt.float32r
    AF = mybir.ActivationFunctionType

    nc = bacc.Bacc("TRN2", target_bir_lowering=False, debug=False, num_devices=8)

    xt_d = nc.dram_tensor("x_t", [NIN, N + 2 * GD], f32, kind="ExternalInput").ap()
    wlt_d = nc.dram_tensor("w_lt", [NIN, 3, H], f32, kind="ExternalInput").ap()
    bl_d = nc.dram_tensor("b_l", [H], f32, kind="ExternalInput").ap()
    wrt_d = nc.dram_tensor("w_rt", [NIN, 3, H], f32, kind="ExternalInput").ap()
    br_d = nc.dram_tensor("b_r", [H], f32, kind="ExternalInput").ap()
    wb_d = nc.dram_tensor("w_bil", [NOUT, H, H], f32, kind="ExternalInput").ap()
    bb_d = nc.dram_tensor("b_bil", [NOUT], f32, kind="ExternalInput").ap()
    out_d = nc.dram_tensor("out", [N, N, NOUT], f32, kind="ExternalOutput").ap()
    # row-block view: [8 blocks, 128 rows, (j,k) interleaved 2048]
    out_v = out_d.rearrange("(t p) n k -> t p (n k)", p=128)

    with tile.TileContext(nc) as tc:
        with (
            tc.tile_pool(name="const", bufs=1) as const,
            tc.tile_pool(name="ps", bufs=7, space="PSUM") as ps,
            tc.tile_pool(name="wps", bufs=1, space="PSUM") as wps_pool,
            tc.tile_pool(name="ob", bufs=4) as ob,
        ):
            # ---- input DMAs, most critical first
            xT_f = const.tile([NIN, N + 2 * GD], f32)
            nc.sync.dma_start(out=xT_f, in_=xt_d)
            wlT_f = const.tile([NIN, 3, H], f32)
            nc.sync.dma_start(out=wlT_f, in_=wlt_d)
            wrT_f = const.tile([NIN, 3, H], f32)
            nc.sync.dma_start(out=wrT_f, in_=wrt_d)

            # ---- PE warmup spinner (fills the pre-input idle window)
            warm = const.tile([128, 256], f32)
            nc.vector.memset(warm, 0.0)
            # dummy ACT ops on a scratch tile: pull the lazy ACT table load
            # to the front without adding deps on `warm`
            actscratch = const.tile([1, 4], f32)
            nc.scalar.activation(actscratch[0:1, 0:2], warm[0:1, 0:2], AF.Relu)
            nc.scalar.activation(actscratch[0:1, 2:4], warm[0:1, 0:2], AF.Identity)
            wps = wps_pool.tile([128, NCH], f32, tag="warm")

            def heartbeat():
                # fp32r matmuls don't count as PE-busy for the HAM clock
                # gate; a small plain-fp32 matmul keeps the PE at 2.4 GHz
                nc.tensor.matmul(
                    wps[:, 0:256], warm[:, 0:128], warm,
                    start=True, stop=True, skip_group_check=True,
                )

            for _ in range(NWARM):
                heartbeat()

            # ---- f32r rounding copies on DVE, critical (x) first
            xT = const.tile([NIN, N + 2 * GD], f32r)
            nc.vector.tensor_copy(xT[:], xT_f[:])
            wlT = const.tile([NIN, 3, H], f32r)
            nc.vector.tensor_copy(wlT[:], wlT_f[:])
            wrT = const.tile([NIN, 3, H], f32r)
            nc.vector.tensor_copy(wrT[:], wrT_f[:])

            # ---- non-critical loads (SWDGE), after the casts in program order
            bl_s = const.tile([H, 1], f32)
            nc.gpsimd.dma_start(out=bl_s, in_=bl_d.unsqueeze(1))
            br_s = const.tile([H, 1], f32)
            nc.gpsimd.dma_start(out=br_s, in_=br_d.unsqueeze(1))
            bb_s = const.tile([128, NOUT], f32)
            nc.gpsimd.dma_start(
                out=bb_s, in_=bb_d.unsqueeze(0).broadcast_to([128, NOUT])
            )
            wb0_f = const.tile([H, H], f32)
            nc.gpsimd.dma_start(out=wb0_f, in_=wb_d[0])
            wb1_f = const.tile([H, H], f32)
            nc.gpsimd.dma_start(out=wb1_f, in_=wb_d[1])
            wb0 = const.tile([H, H], f32r)
            nc.scalar.copy(wb0[:], wb0_f[:])
            wb1 = const.tile([H, H], f32r)
            nc.scalar.copy(wb1[:], wb1_f[:])

            hlT = const.tile([H, N], f32r)
            hrT = const.tile([H, N], f32r)

            def h_chunk(dst, wt, bias, s1, j0):
                # chunk 1 is shift_down (src col i-1) for xl, shift_up (i+1) for xr
                ph = ps.tile([128, NCH], f32, tag="ps")
                for c, s in ((0, 0), (1, s1), (2, -s1)):
                    nc.tensor.matmul(
                        ph[:, :],
                        wt[:, c, :],
                        xT[:, GD + j0 + s : GD + j0 + s + NCH],
                        start=(c == 0), stop=(c == 2),
                    )
                nc.scalar.activation(
                    dst[:, j0 : j0 + NCH], ph[:, :], AF.Relu,
                    bias=bias[:, 0:1], scale=1.0,
                )

            tT0 = const.tile([H, N], f32r)
            tT1 = const.tile([H, N], f32r)

            def t_chunk(wb, tT, j0):
                pt = ps.tile([128, NCH], f32, tag="ps")
                nc.tensor.matmul(
                    pt[:, :], wb, hlT[:, j0 : j0 + NCH], start=True, stop=True
                )
                nc.vector.tensor_copy(tT[:, j0 : j0 + NCH], pt[:, :])

            def out_half(iblk, j0, hb=False):
                if hb:
                    heartbeat()
                ohalf = ob.tile([128, 2 * NCH], f32, tag="ob")
                for k, tT in ((0, tT0), (1, tT1)):
                    po = ps.tile([128, NCH], f32, tag="ps")
                    nc.tensor.matmul(
                        po[:, :],
                        tT[:, iblk * 128 : (iblk + 1) * 128],
                        hrT[:, j0 : j0 + NCH],
                        start=True, stop=True,
                    )
                    dst = ohalf[:, k : 2 * NCH : 2]
                    if k == 0:
                        nc.vector.tensor_scalar_add(dst, po[:, :], bb_s[:, 0:1])
                    else:
                        nc.scalar.activation(
                            dst, po[:, :], AF.Identity, bias=bb_s[:, 1:2], scale=1.0
                        )
                nc.sync.dma_start(
                    out=out_v[iblk][:, 2 * j0 : 2 * j0 + 2 * NCH], in_=ohalf
                )

            # emission order: earliest first output DMA, PE kept dense
            h_chunk(hlT, wlT, bl_s, -1, 0)
            h_chunk(hrT, wrT, br_s, +1, 0)
            heartbeat()
            h_chunk(hlT, wlT, bl_s, -1, NCH)   # PE filler while relu0 lands
            h_chunk(hrT, wrT, br_s, +1, NCH)
            heartbeat()
            t_chunk(wb0, tT0, 0)
            t_chunk(wb1, tT1, 0)
            for iblk in range(4):
                out_half(iblk, 0, hb=(iblk % 2 == 1))
            t_chunk(wb0, tT0, NCH)
            t_chunk(wb1, tT1, NCH)
            for iblk in range(4):
                out_half(iblk, NCH, hb=(iblk % 2 == 1))
            for iblk in range(4, 8):
                out_half(iblk, 0, hb=True)
                out_half(iblk, NCH)

    nc.finalize()
    return nc


def make_in_maps(x_l, W_l, b_l, W_r, b_r, W_bil, b_bil):
    # host-side layout: W chunks to lhsT [f=64, chunk, h], x to [64, N] with
    # zero guard columns
    def w_chunks(W):
        return np.ascontiguousarray(
            np.asarray(W, np.float32).reshape(H, 3, NIN).transpose(2, 1, 0)
        )

    x_l = np.asarray(x_l, np.float32)
    xt = np.zeros((B, NIN, N + 2 * GD), np.float32)
    xt[:, :, GD : GD + N] = x_l.transpose(0, 2, 1)

    com = {
        "w_lt": w_chunks(W_l),
        "b_l": np.ascontiguousarray(b_l, np.float32),
        "w_rt": w_chunks(W_r),
        "b_r": np.ascontiguousarray(b_r, np.float32),
        "w_bil": np.ascontiguousarray(W_bil, np.float32),
        "b_bil": np.ascontiguousarray(b_bil, np.float32),
    }
    return [{"x_t": np.ascontiguousarray(xt[c]), **com} for c in range(B)]


def kernel(x_l, W_l, b_l, W_r, b_r, W_bil, b_bil):
    from concourse import bass_utils

    if "nc" not in _cached:
        _cached["nc"] = _build()
    nc = _cached["nc"]

    in_maps = make_in_maps(x_l, W_l, b_l, W_r, b_r, W_bil, b_bil)
    res = bass_utils.run_bass_kernel_spmd(nc, in_maps, core_ids=list(range(B)))
    return np.stack([res.results[c]["out"] for c in range(B)], axis=0)
